# revision 31
# baseline (speedup 1.0000x reference)
"""Trainium2 Bass kernel for nn_MatchLoss.

Reference computation:
    an, bn, cn = l1_normalize(a|b|c, dim=C)        # per (b, h, w) column
    sim_ab = einsum('bchw,bcij->bhwij', an, bn)
    sim_ac = einsum('bchw,bcij->bhwij', an, cn)
    out = mean(|sim_ac - sim_ab|)                   # scalar

Restructure (per batch, hw -> 4096):
    sim_ac - sim_ab = diag(1/na) @ (a^T @ D),  D = c*diag(1/nc) - b*diag(1/nb)
    loss_part = sum_q (1/na[q]) * sum_p |(a^T D)[q, p]|

Sharding: 8 cores = 2 batches x 4 slices of the p axis.  Each core gets
full `a` for its batch plus a 1024-column slice of b and c (packed as one
dram tensor [b0|c0|b1|c1] in 512-col chunks), computes 1/na-scaled
rowsums of |a^T D| into a (128 x 32) partial; host sums the partials.

v2 schedule (vs the 34.7us baseline):
  * all inputs are fp8e4 host-side (half the DMA bytes); the main matmul
    runs mixed fp8 lhsT x bf16 rhs (hw-validated), D stays bf16
  * |b|,|c|,|a| via DVE tensor_scalar bitwise-AND 0x7f7f on a uint16
    bitcast (4x DVE mode, ~0.26ns/elem) -- kills the ACT abs prepass
    that blocked the ACT reduce lane until ~10.7us
  * na matmuls run BEFORE the main loop (absA is ready early now), into
    spare columns of the nbc PSUM tile, so 1/na is ready mid-loop and
    the result can stream out in two halves
  * first EARLY tiles run chunk-granular (512-col) matmul+reduce against
    D-chunk-0 so the ACT lane starts ~2us before D is complete
  * reduces stay on the only two PSUM-capable engines: ACT
    activation(Abs)+accum and DVE tensor_reduce, rebalanced 15/13 plus
    the early chunk pairs
  * PSUM: two 2-deep rings (A/B) alternate tiles; ring B reuses the
    head-pool banks after the norm chain retires
"""

import numpy as np

try:
    import concourse.bacc as bacc
    import concourse.tile as tile
    import concourse.mybir as mybir
    from concourse import bass_utils
except ImportError:  # pragma: no cover - fallback for bare containers
    import sys

    sys.path.insert(0, "/opt/trn_rl_repo")
    import concourse.bacc as bacc
    import concourse.tile as tile
    import concourse.mybir as mybir
    from concourse import bass_utils

B, C, H, W = 2, 128, 64, 64
HW = H * W              # 4096 (q axis, and full p axis)
N_CORES = 8
PSL = HW // 4           # 1024: per-core p-slice
QT = 128                # q tile (partition dim of PSUM result)
NQT = HW // QT          # 32 q tiles
CH = 512                # matmul moving chunk (one PSUM bank of fp32)
NCHK = PSL // CH        # 2 chunks per core

_F32 = mybir.dt.float32
_BF16 = mybir.dt.bfloat16
_FP8 = mybir.dt.float8e4
_U16 = mybir.dt.uint16
_AX = mybir.AxisListType
_AF = mybir.ActivationFunctionType
_OP = mybir.AluOpType

# (chunk-granular EARLY tiles were tried and backfired: they pin the two
# ring-A buffers until the late DVE chunk-1 reduces, stalling the ring)


def _assign_engines():
    """Lane assignment for the full-tile reduces: alternate ACT/DVE 16/16."""
    return {t: ("A" if t % 2 == 0 else "D") for t in range(NQT)}


def _emit(tc, a_d, bc_d, oh_d, id_d, o_d):
    nc = tc.nc
    import contextlib

    lanes = _assign_engines()

    with contextlib.ExitStack() as ctx:
        ctx.enter_context(
            nc.allow_low_precision(
                reason="fp8/bf16 matmul inputs; accumulation stays fp32"
            )
        )
        sb = ctx.enter_context(tc.tile_pool(name="sb", bufs=1))

        A8a = sb.tile([C, HW // 2], _FP8)      # a cols 0:2048
        A8b = sb.tile([C, HW // 2], _FP8)      # a cols 2048:4096
        bc8 = sb.tile([C, 2 * PSL], _FP8)      # [b0|c0|b1|c1] 512-col chunks
        absBC = sb.tile([C, 2 * PSL], _FP8)
        absA8a = sb.tile([C, HW // 2], _FP8)
        absA8b = sb.tile([C, HW // 2], _FP8)
        D = sb.tile([C, PSL], _BF16)
        t1a = sb.tile([C, CH], _BF16)
        t2a = sb.tile([C, CH], _BF16)
        t1b = sb.tile([C, CH], _BF16)
        t2b = sb.tile([C, CH], _BF16)
        ones8 = sb.tile([C, 1], _FP8)
        zeros_w = sb.tile([C, 256], _BF16)
        rr = sb.tile([C, 16], _BF16)           # 1/norm, partition-major
        rrT = sb.tile([16, C], _BF16)          # all 16 block norms as rows
        onehots = sb.tile([16, 16 * QT], _BF16)  # K=16 row selectors (DMA'd const)
        ident = sb.tile([C, C], _BF16)         # identity for PE transpose
        rna = sb.tile([C, NQT], _F32)
        rs_d = sb.tile([C, NQT], _F32)
        rs_a = sb.tile([C, NQT], _F32)
        sum1 = sb.tile([C, NQT], _F32)
        res = sb.tile([C, NQT], _F32)
        trash_a = sb.tile([C, PSL], _BF16)

        # --- t=0: DMA issue + memsets + PE warmup -------------------------
        # bc/ident/oh ride the sync queue's HWDGE (625ns serial apiece);
        # the two big a halves go through Pool's SWDGE instead, which skips
        # the HWDGE bottleneck entirely.  The A8a descriptor-gen is floored
        # past the small transfers so it does not butt into ident/oh on the
        # (FIFO) DMA bus.
        nc.sync.dma_start(bc8[:, 0:1024], bc_d[:, 0:1024])
        nc.sync.dma_start(bc8[:, 1024:2048], bc_d[:, 1024:2048])
        nc.sync.dma_start(ident[:], id_d[:, :])
        nc.sync.dma_start(onehots[:], oh_d[:, 0 : 16 * QT])

        # memsets on Pool (cheap there; keeps DVE free for the chain)
        nc.gpsimd.memset(zeros_w[:], 0.0)
        nc.gpsimd.memset(ones8[:], 1.0)
        nc.gpsimd.memset(rs_d[:], 0.0)
        nc.gpsimd.memset(rs_a[:], 0.0)

        # the two big a halves go through Pool's SWDGE: they skip the
        # HWDGE queue (625ns per transfer, serialized) and land ~1.6us
        # earlier, which lets both ACT |a| passes finish before the first
        # reduce needs the ACT lane
        nc.gpsimd.dma_start(A8a[:], a_d[:, 0:2048])
        nc.gpsimd.dma_start(A8b[:], a_d[:, 2048:4096])

        with tc.tile_pool(name="warm_ps", bufs=1, space="PSUM") as warm_ps:
            warm = warm_ps.tile([C, 256], _F32)
            nc.tensor.matmul(
                warm[:], lhsT=zeros_w[:, 0:QT], rhs=zeros_w[:],
                start=True, stop=True,
            )

        # Ring A opens before the head pools: pools release in LIFO order,
        # and the head pools close mid-emission while ring A lives on.
        ring_a = ctx.enter_context(tc.tile_pool(name="m_psA", bufs=2, space="PSUM"))

        head_ctx = contextlib.ExitStack()
        nbc_ps = head_ctx.enter_context(tc.tile_pool(name="nbc_ps", bufs=1, space="PSUM"))
        rrt_ps = head_ctx.enter_context(tc.tile_pool(name="rrt_ps", bufs=1, space="PSUM"))
        bcst_ps = head_ctx.enter_context(tc.tile_pool(name="bcst_ps", bufs=2, space="PSUM"))

        # warm the Abs activation table, then |a| on ACT in DMA-piece halves
        # (Pool cannot run bitwise tensor_scalar through codegen; ACT is idle
        # until the first reduce anyway)
        nc.scalar.activation(trash_a[:, 0:1], ones8[:], _AF.Abs, bias=0.0)
        nc.scalar.activation(absA8a[:], A8a[:], _AF.Abs, bias=0.0)

        # nbc col layout: cols 0..15 = b/c block norms (j*8+u, u<4: b block
        # u, u>=4: c block u-4); cols 16..47 = na per q-tile.
        nbc = nbc_ps.tile([C, 48], _F32)

        def absbc(j):
            """|b_j|,|c_j| via bitwise AND on uint16 pairs (4x DVE mode)."""
            base = 1024 * j
            nc.vector.tensor_scalar(
                out=absBC[:, base : base + 1024].bitcast(_U16),
                in0=bc8[:, base : base + 1024].bitcast(_U16),
                scalar1=0x7F7F, scalar2=None, op0=_OP.bitwise_and,
            )

        def norm_mms(j):
            base = 1024 * j
            for u in range(8):
                nc.tensor.matmul(
                    nbc[:, j * 8 + u : j * 8 + u + 1],
                    lhsT=absBC[:, base + u * QT : base + (u + 1) * QT],
                    rhs=ones8[:],
                    start=True, stop=True,
                )

        def bcast(j):
            """broadcast 1/nb, 1/nc rows across partitions via K=16 matmul
            with one-hot selector weights (rows j*8+u of rrT)"""
            rb_bc = bcst_ps.tile([C, CH], _F32, tag="bcst")
            rc_bc = bcst_ps.tile([C, CH], _F32, tag="bcst")
            for u in range(4):
                k = j * 8 + u
                nc.tensor.matmul(
                    rb_bc[:, u * QT : (u + 1) * QT],
                    lhsT=onehots[:, k * QT : (k + 1) * QT],
                    rhs=rrT[:],
                    start=True, stop=True,
                )
            for u in range(4):
                k = j * 8 + 4 + u
                nc.tensor.matmul(
                    rc_bc[:, u * QT : (u + 1) * QT],
                    lhsT=onehots[:, k * QT : (k + 1) * QT],
                    rhs=rrT[:],
                    start=True, stop=True,
                )
            return rb_bc, rc_bc

        # --- head ---------------------------------------------------------
        # DVE: absbc0, absbc1, recip0, cpy0, recip1, cpy1, t1a, t2a, sub0,
        #      t1b, t2b, sub1, rna  (floors keep the list scheduler honest)
        # PE:  warm, norm0, rrt0, norm1, bcast0, rrt1, bcast1, c0-prefetch,
        #      na, main
        absbc(0)
        absbc(1)
        norm_mms(0)
        norm_mms(1)
        with tc.tile_wait_until(0.0040):
            nc.vector.reciprocal(rr[:], nbc[:, 0:16])
        rrt = rrt_ps.tile([16, C], _BF16, tag="rrt")
        nc.tensor.matmul(
            rrt[:], lhsT=rr[:], rhs=ident[:],
            start=True, stop=True, is_transpose=True,
        )
        with tc.tile_wait_until(0.0044):
            nc.vector.tensor_copy(out=rrT[:], in_=rrt[:])

        rb0, rc0 = bcast(0)
        rb1, rc1 = bcast(1)
        with tc.tile_wait_until(0.0048):
            nc.vector.tensor_tensor(out=t1a[:], in0=bc8[:, 0:CH], in1=rb0[:], op=_OP.mult)
        with tc.tile_wait_until(0.0054):
            nc.vector.tensor_tensor(out=t2a[:], in0=bc8[:, CH:1024], in1=rc0[:], op=_OP.mult)
        with tc.tile_wait_until(0.0058):
            nc.vector.tensor_tensor(out=D[:, 0:CH], in0=t1a[:], in1=t2a[:], op=_OP.subtract)
        # |a| second half via DVE bitwise-AND (4x mode, ~330ns): A8b's data
        # is long since landed here, and the na matmuls only need it by
        # ~9us, so this sits behind D chunk 0 without touching the D1 path
        with tc.tile_wait_until(0.0062):
            nc.vector.tensor_scalar(
                out=absA8b[:].bitcast(_U16), in0=A8b[:].bitcast(_U16),
                scalar1=0x7F7F, scalar2=None, op0=_OP.bitwise_and,
            )
        with tc.tile_wait_until(0.0068):
            nc.vector.tensor_tensor(out=t1b[:], in0=bc8[:, 1024 : 1024 + CH], in1=rb1[:], op=_OP.mult)
        with tc.tile_wait_until(0.0074):
            nc.vector.tensor_tensor(out=t2b[:], in0=bc8[:, 1024 + CH : 2048], in1=rc1[:], op=_OP.mult)
        with tc.tile_wait_until(0.0080):
            nc.vector.tensor_tensor(out=D[:, CH:PSL], in0=t1b[:], in1=t2b[:], op=_OP.subtract)

        def a_blk(t):
            src = A8a if t < 16 else A8b
            tt = t % 16
            return src[:, tt * QT : (tt + 1) * QT]

        def absa_blk(t):
            src = absA8a if t < 16 else absA8b
            tt = t % 16
            return src[:, tt * QT : (tt + 1) * QT]

        # tile-0 chunk 0 goes to a recycled bcst-pool tile (same [C,512]
        # f32 shape, no extra banks) so ACT can chunk-reduce it ~8.5us,
        # well before D1;
        # its chunk 1 lands in a ring-A tile's upper half later.  Tile 1
        # gets a plain c0 prefetch.
        m0c0 = bcst_ps.tile([C, CH], _F32, tag="bcst")
        nc.tensor.matmul(
            m0c0[:], lhsT=a_blk(0), rhs=D[:, 0:CH],
            start=True, stop=True,
        )
        prefetch = {}
        for t in (1,):
            M = ring_a.tile([C, PSL], _F32, tag="mA")
            nc.tensor.matmul(
                M[:, 0:CH], lhsT=a_blk(t), rhs=D[:, 0:CH],
                start=True, stop=True,
            )
            prefetch[t] = M

        # na matmuls: PE runs them in its |a|-gated window before D1
        for t in range(NQT):
            nc.tensor.matmul(
                nbc[:, 16 + t : 17 + t], lhsT=absa_blk(t), rhs=ones8[:],
                start=True, stop=True,
            )
        with tc.tile_wait_until(0.0097):
            nc.vector.reciprocal(rna[:], nbc[:, 16:48])

        # early ACT reduce of tile-0 chunk 0 (c0 -> rs_a[0], c1 -> rs_d[0])
        nc.scalar.activation(
            trash_a[:, 0:CH], m0c0[:], _AF.Abs, bias=0.0,
            accum_out=rs_a[:, 0:1],
        )

        head_ctx.close()
        ring_b = ctx.enter_context(tc.tile_pool(name="m_psB", bufs=2, space="PSUM"))

        def emit_reduce(eng, m_ap, rs_tile, t):
            w = m_ap.shape[-1]
            if eng == "D":
                nc.vector.tensor_reduce(
                    out=rs_tile[:, t : t + 1], in_=m_ap, axis=_AX.X,
                    op=_OP.add, apply_absolute_value=True,
                )
            elif eng == "A":
                nc.scalar.activation(
                    trash_a[:, 0:w], m_ap, _AF.Abs, bias=0.0,
                    accum_out=rs_tile[:, t : t + 1],
                )

        # --- main loop ----------------------------------------------------
        for t in range(NQT):
            if t == 0:
                M = ring_a.tile([C, PSL], _F32, tag="mA")
                nc.tensor.matmul(
                    M[:, CH:PSL], lhsT=a_blk(0), rhs=D[:, CH:PSL],
                    start=True, stop=True,
                )
                nc.scalar.activation(
                    trash_a[:, 0:CH], M[:, CH:PSL], _AF.Abs, bias=0.0,
                    accum_out=rs_d[:, 0:1],
                )
                continue
            if t in prefetch:
                M = prefetch[t]
            else:
                pool, tag = (ring_a, "mA") if t % 4 in (0, 1) else (ring_b, "mB")
                M = pool.tile([C, PSL], _F32, tag=tag)
                nc.tensor.matmul(
                    M[:, 0:CH], lhsT=a_blk(t), rhs=D[:, 0:CH],
                    start=True, stop=True,
                )
            nc.tensor.matmul(
                M[:, CH:PSL], lhsT=a_blk(t), rhs=D[:, CH:PSL],
                start=True, stop=True,
            )
            e = lanes[t]
            emit_reduce(e, M[:], rs_d if e == "D" else rs_a, t)

            if t == 15:
                # first-half result: combine + scale + store while the
                # second half is still reducing
                nc.vector.tensor_tensor(out=sum1[:, 0:16], in0=rs_d[:, 0:16], in1=rs_a[:, 0:16], op=_OP.add)
                nc.vector.tensor_tensor(out=res[:, 0:16], in0=sum1[:, 0:16], in1=rna[:, 0:16], op=_OP.mult)
                nc.sync.dma_start(o_d[:, 0:16], res[:, 0:16])

        # --- tail ---------------------------------------------------------
        nc.vector.tensor_tensor(out=sum1[:, 16:32], in0=rs_d[:, 16:32], in1=rs_a[:, 16:32], op=_OP.add)
        nc.vector.tensor_tensor(out=res[:, 16:32], in0=sum1[:, 16:32], in1=rna[:, 16:32], op=_OP.mult)
        nc.sync.dma_start(o_d[:, 16:32], res[:, 16:32])


def _decls(nc):
    a_d = nc.dram_tensor("a_full", (C, HW), _FP8, kind="ExternalInput").ap()
    bc_d = nc.dram_tensor("bc", (C, 2 * PSL), _FP8, kind="ExternalInput").ap()
    oh_d = nc.dram_tensor("oh", (16, 16 * QT), _BF16, kind="ExternalInput").ap()
    id_d = nc.dram_tensor("ident", (C, C), _BF16, kind="ExternalInput").ap()
    o_d = nc.dram_tensor("out", (C, NQT), _F32, kind="ExternalOutput").ap()
    return a_d, bc_d, oh_d, id_d, o_d


def _build():
    nc = bacc.Bacc(
        "TRN2", target_bir_lowering=False, debug=False, num_devices=N_CORES
    )
    args = _decls(nc)
    with tile.TileContext(nc) as tc:
        _emit(tc, *args)
    nc.finalize()
    return nc


def build_single():
    """Single-core module for TimelineSim tracing."""
    nc = bacc.Bacc("TRN2", target_bir_lowering=False, debug=False)
    args = _decls(nc)
    with tile.TileContext(nc) as tc:
        _emit(tc, *args)
    nc.finalize()
    return nc


_NC_CACHE = {}


def _get_nc():
    if "nc" not in _NC_CACHE:
        _NC_CACHE["nc"] = _build()
    return _NC_CACHE["nc"]


def _fp8(x):
    import ml_dtypes

    return np.ascontiguousarray(x.astype(ml_dtypes.float8_e4m3))


def _bf16(x):
    import ml_dtypes

    return np.ascontiguousarray(x.astype(ml_dtypes.bfloat16))


def _in_maps(a, b, c):
    a = np.asarray(a, dtype=np.float32).reshape(B, C, HW)
    b = np.asarray(b, dtype=np.float32).reshape(B, C, HW)
    c = np.asarray(c, dtype=np.float32).reshape(B, C, HW)
    maps = []
    for core in range(N_CORES):
        bi, pi = divmod(core, 4)
        s0 = pi * PSL
        bc = np.concatenate(
            [
                b[bi, :, s0 : s0 + CH],
                c[bi, :, s0 : s0 + CH],
                b[bi, :, s0 + CH : s0 + PSL],
                c[bi, :, s0 + CH : s0 + PSL],
            ],
            axis=1,
        )
        maps.append(
            {
                "a_full": _fp8(a[bi]),
                "bc": _fp8(bc),
                "oh": _bf16(_onehot_const()),
                "ident": _bf16(np.eye(C, dtype=np.float32)),
            }
        )
    return maps


def _onehot_const():
    oh = np.zeros((16, 16 * QT), dtype=np.float32)
    for u in range(16):
        oh[u, u * QT : (u + 1) * QT] = 1.0
    return oh


def kernel(a, b, c):
    nc = _get_nc()
    res = bass_utils.run_bass_kernel_spmd(
        nc, _in_maps(a, b, c), core_ids=list(range(N_CORES))
    )
    total = np.float64(0.0)
    for core in range(N_CORES):
        total += np.sum(res.results[core]["out"], dtype=np.float64)
    return np.float32(total / (B * HW * HW))


# revision 32
# speedup vs baseline: 1.0465x; 1.0465x over previous
"""Trainium2 Bass kernel for nn_MatchLoss.

Reference computation:
    an, bn, cn = l1_normalize(a|b|c, dim=C)        # per (b, h, w) column
    sim_ab = einsum('bchw,bcij->bhwij', an, bn)
    sim_ac = einsum('bchw,bcij->bhwij', an, cn)
    out = mean(|sim_ac - sim_ab|)                   # scalar

Restructure (per batch, hw -> 4096):
    sim_ac - sim_ab = diag(1/na) @ (a^T @ D),  D = c*diag(1/nc) - b*diag(1/nb)
    loss_part = sum_q (1/na[q]) * sum_p |(a^T D)[q, p]|

Sharding: 8 cores = 2 batches x 4 slices of the p axis.  Each core gets
full `a` for its batch plus a 1024-column slice of b and c (packed as one
dram tensor [b0|c0|b1|c1] in 512-col chunks), computes 1/na-scaled
rowsums of |a^T D| into a (128 x 32) partial; host sums the partials.

v2 schedule (vs the 34.7us baseline):
  * all inputs are fp8e4 host-side (half the DMA bytes); the main matmul
    runs mixed fp8 lhsT x bf16 rhs (hw-validated), D stays bf16
  * |b|,|c|,|a| via DVE tensor_scalar bitwise-AND 0x7f7f on a uint16
    bitcast (4x DVE mode, ~0.26ns/elem) -- kills the ACT abs prepass
    that blocked the ACT reduce lane until ~10.7us
  * na matmuls run BEFORE the main loop (absA is ready early now), into
    spare columns of the nbc PSUM tile, so 1/na is ready mid-loop and
    the result can stream out in two halves
  * first EARLY tiles run chunk-granular (512-col) matmul+reduce against
    D-chunk-0 so the ACT lane starts ~2us before D is complete
  * reduces stay on the only two PSUM-capable engines: ACT
    activation(Abs)+accum and DVE tensor_reduce, rebalanced 15/13 plus
    the early chunk pairs
  * PSUM: two 2-deep rings (A/B) alternate tiles; ring B reuses the
    head-pool banks after the norm chain retires
"""

import numpy as np

try:
    import concourse.bacc as bacc
    import concourse.tile as tile
    import concourse.mybir as mybir
    from concourse import bass_utils
except ImportError:  # pragma: no cover - fallback for bare containers
    import sys

    sys.path.insert(0, "/opt/trn_rl_repo")
    import concourse.bacc as bacc
    import concourse.tile as tile
    import concourse.mybir as mybir
    from concourse import bass_utils

B, C, H, W = 2, 128, 64, 64
HW = H * W              # 4096 (q axis, and full p axis)
N_CORES = 8
PSL = HW // 4           # 1024: per-core p-slice
QT = 128                # q tile (partition dim of PSUM result)
NQT = HW // QT          # 32 q tiles
CH = 512                # matmul moving chunk (one PSUM bank of fp32)
NCHK = PSL // CH        # 2 chunks per core

_F32 = mybir.dt.float32
_BF16 = mybir.dt.bfloat16
_FP8 = mybir.dt.float8e4
_U16 = mybir.dt.uint16
_AX = mybir.AxisListType
_AF = mybir.ActivationFunctionType
_OP = mybir.AluOpType

# (chunk-granular EARLY tiles were tried and backfired: they pin the two
# ring-A buffers until the late DVE chunk-1 reduces, stalling the ring)


def _assign_engines():
    """Lane assignment for the full-tile reduces: alternate ACT/DVE 16/16."""
    return {t: ("A" if t % 2 == 0 else "D") for t in range(NQT)}


def _emit(tc, a_d, bc_d, oh_d, id_d, o_d):
    nc = tc.nc
    import contextlib

    lanes = _assign_engines()

    with contextlib.ExitStack() as ctx:
        ctx.enter_context(
            nc.allow_low_precision(
                reason="fp8/bf16 matmul inputs; accumulation stays fp32"
            )
        )
        sb = ctx.enter_context(tc.tile_pool(name="sb", bufs=1))

        A8a = sb.tile([C, HW // 2], _FP8)      # a cols 0:2048
        A8b = sb.tile([C, HW // 2], _FP8)      # a cols 2048:4096
        bc8 = sb.tile([C, 2 * PSL], _FP8)      # [b0|c0|b1|c1] 512-col chunks
        absBC = sb.tile([C, 2 * PSL], _FP8)
        absA8a = sb.tile([C, HW // 2], _FP8)
        absA8b = sb.tile([C, HW // 2], _FP8)
        D = sb.tile([C, PSL], _BF16)
        t1a = sb.tile([C, CH], _BF16)
        t2a = sb.tile([C, CH], _BF16)
        t1b = sb.tile([C, CH], _BF16)
        t2b = sb.tile([C, CH], _BF16)
        ones8 = sb.tile([C, 1], _FP8)
        zeros_w = sb.tile([C, 256], _BF16)
        rr = sb.tile([C, 16], _BF16)           # 1/norm, partition-major
        rrT = sb.tile([16, C], _BF16)          # all 16 block norms as rows
        onehots = sb.tile([16, 16 * QT], _BF16)  # K=16 row selectors (DMA'd const)
        ident = sb.tile([C, C], _BF16)         # identity for PE transpose
        rna = sb.tile([C, NQT], _F32)
        rs_d = sb.tile([C, NQT], _F32)
        rs_a = sb.tile([C, NQT], _F32)
        sum1 = sb.tile([C, NQT], _F32)
        res = sb.tile([C, NQT], _F32)
        trash_a = sb.tile([C, PSL], _BF16)

        # --- t=0: DMA issue + memsets + PE warmup -------------------------
        # bc/ident/oh ride the sync queue's HWDGE (625ns serial apiece);
        # the two big a halves go through Pool's SWDGE instead, which skips
        # the HWDGE bottleneck entirely.  The A8a descriptor-gen is floored
        # past the small transfers so it does not butt into ident/oh on the
        # (FIFO) DMA bus.
        nc.sync.dma_start(bc8[:, 0:1024], bc_d[:, 0:1024])
        nc.sync.dma_start(bc8[:, 1024:2048], bc_d[:, 1024:2048])
        nc.sync.dma_start(ident[:], id_d[:, :])
        nc.sync.dma_start(onehots[:], oh_d[:, 0 : 16 * QT])

        # memsets on Pool (cheap there; keeps DVE free for the chain)
        nc.gpsimd.memset(zeros_w[:], 0.0)
        nc.gpsimd.memset(ones8[:], 1.0)
        nc.gpsimd.memset(rs_d[:], 0.0)
        nc.gpsimd.memset(rs_a[:], 0.0)

        # the two big a halves go through Pool's SWDGE: they skip the
        # HWDGE queue (625ns per transfer, serialized) and land ~1.6us
        # earlier, which lets both ACT |a| passes finish before the first
        # reduce needs the ACT lane
        nc.gpsimd.dma_start(A8a[:], a_d[:, 0:2048])
        nc.gpsimd.dma_start(A8b[:], a_d[:, 2048:4096])

        with tc.tile_pool(name="warm_ps", bufs=1, space="PSUM") as warm_ps:
            warm = warm_ps.tile([C, 256], _F32)
            nc.tensor.matmul(
                warm[:], lhsT=zeros_w[:, 0:QT], rhs=zeros_w[:],
                start=True, stop=True,
            )

        # Ring A opens before the head pools: pools release in LIFO order,
        # and the head pools close mid-emission while ring A lives on.
        ring_a = ctx.enter_context(tc.tile_pool(name="m_psA", bufs=2, space="PSUM"))

        head_ctx = contextlib.ExitStack()
        nbc_ps = head_ctx.enter_context(tc.tile_pool(name="nbc_ps", bufs=1, space="PSUM"))
        rrt_ps = head_ctx.enter_context(tc.tile_pool(name="rrt_ps", bufs=1, space="PSUM"))
        bcst_ps = head_ctx.enter_context(tc.tile_pool(name="bcst_ps", bufs=2, space="PSUM"))

        # warm the Abs activation table, then |a| on ACT in DMA-piece halves
        # (Pool cannot run bitwise tensor_scalar through codegen; ACT is idle
        # until the first reduce anyway)
        nc.scalar.activation(trash_a[:, 0:1], ones8[:], _AF.Abs, bias=0.0)
        nc.scalar.activation(absA8a[:], A8a[:], _AF.Abs, bias=0.0)

        # nbc col layout: cols 0..15 = b/c block norms (j*8+u, u<4: b block
        # u, u>=4: c block u-4); cols 16..47 = na per q-tile.
        nbc = nbc_ps.tile([C, 48], _F32)

        def absbc(j):
            """|b_j|,|c_j| via bitwise AND on uint16 pairs (4x DVE mode)."""
            base = 1024 * j
            nc.vector.tensor_scalar(
                out=absBC[:, base : base + 1024].bitcast(_U16),
                in0=bc8[:, base : base + 1024].bitcast(_U16),
                scalar1=0x7F7F, scalar2=None, op0=_OP.bitwise_and,
            )

        def norm_mms(j):
            base = 1024 * j
            for u in range(8):
                nc.tensor.matmul(
                    nbc[:, j * 8 + u : j * 8 + u + 1],
                    lhsT=absBC[:, base + u * QT : base + (u + 1) * QT],
                    rhs=ones8[:],
                    start=True, stop=True,
                )

        def bcast(j):
            """broadcast 1/nb, 1/nc rows across partitions via K=16 matmul
            with one-hot selector weights (rows j*8+u of rrT)"""
            rb_bc = bcst_ps.tile([C, CH], _F32, tag="bcst")
            rc_bc = bcst_ps.tile([C, CH], _F32, tag="bcst")
            for u in range(4):
                k = j * 8 + u
                nc.tensor.matmul(
                    rb_bc[:, u * QT : (u + 1) * QT],
                    lhsT=onehots[:, k * QT : (k + 1) * QT],
                    rhs=rrT[:],
                    start=True, stop=True,
                )
            for u in range(4):
                k = j * 8 + 4 + u
                nc.tensor.matmul(
                    rc_bc[:, u * QT : (u + 1) * QT],
                    lhsT=onehots[:, k * QT : (k + 1) * QT],
                    rhs=rrT[:],
                    start=True, stop=True,
                )
            return rb_bc, rc_bc

        # --- head ---------------------------------------------------------
        # DVE: absbc0, absbc1, recip0, cpy0, recip1, cpy1, t1a, t2a, sub0,
        #      t1b, t2b, sub1, rna  (floors keep the list scheduler honest)
        # PE:  warm, norm0, rrt0, norm1, bcast0, rrt1, bcast1, c0-prefetch,
        #      na, main
        absbc(0)
        absbc(1)
        norm_mms(0)
        norm_mms(1)
        with tc.tile_wait_until(0.0040):
            nc.vector.reciprocal(rr[:], nbc[:, 0:16])
        rrt = rrt_ps.tile([16, C], _BF16, tag="rrt")
        nc.tensor.matmul(
            rrt[:], lhsT=rr[:], rhs=ident[:],
            start=True, stop=True, is_transpose=True,
        )
        with tc.tile_wait_until(0.0044):
            nc.vector.tensor_copy(out=rrT[:], in_=rrt[:])

        rb0, rc0 = bcast(0)
        rb1, rc1 = bcast(1)
        with tc.tile_wait_until(0.0048):
            nc.vector.tensor_tensor(out=t1a[:], in0=bc8[:, 0:CH], in1=rb0[:], op=_OP.mult)
        with tc.tile_wait_until(0.0054):
            nc.vector.tensor_tensor(out=t2a[:], in0=bc8[:, CH:1024], in1=rc0[:], op=_OP.mult)
        with tc.tile_wait_until(0.0058):
            nc.vector.tensor_tensor(out=D[:, 0:CH], in0=t1a[:], in1=t2a[:], op=_OP.subtract)
        # |a| second half via DVE bitwise-AND (4x mode, ~330ns): A8b's data
        # is long since landed here, and the na matmuls only need it by
        # ~9us, so this sits behind D chunk 0 without touching the D1 path
        with tc.tile_wait_until(0.0062):
            nc.vector.tensor_scalar(
                out=absA8b[:].bitcast(_U16), in0=A8b[:].bitcast(_U16),
                scalar1=0x7F7F, scalar2=None, op0=_OP.bitwise_and,
            )
        with tc.tile_wait_until(0.0068):
            nc.vector.tensor_tensor(out=t1b[:], in0=bc8[:, 1024 : 1024 + CH], in1=rb1[:], op=_OP.mult)
        with tc.tile_wait_until(0.0074):
            nc.vector.tensor_tensor(out=t2b[:], in0=bc8[:, 1024 + CH : 2048], in1=rc1[:], op=_OP.mult)
        with tc.tile_wait_until(0.0080):
            nc.vector.tensor_tensor(out=D[:, CH:PSL], in0=t1b[:], in1=t2b[:], op=_OP.subtract)

        def a_blk(t):
            src = A8a if t < 16 else A8b
            tt = t % 16
            return src[:, tt * QT : (tt + 1) * QT]

        def absa_blk(t):
            src = absA8a if t < 16 else absA8b
            tt = t % 16
            return src[:, tt * QT : (tt + 1) * QT]

        # tile-0 chunk 0 goes to a recycled bcst-pool tile (same [C,512]
        # f32 shape, no extra banks) so ACT can chunk-reduce it ~8.5us,
        # well before D1;
        # its chunk 1 lands in a ring-A tile's upper half later.  Tile 1
        # gets a plain c0 prefetch.
        m0c0 = bcst_ps.tile([C, CH], _F32, tag="bcst")
        nc.tensor.matmul(
            m0c0[:], lhsT=a_blk(0), rhs=D[:, 0:CH],
            start=True, stop=True,
        )
        prefetch = {}
        for t in (1,):
            M = ring_a.tile([C, PSL], _F32, tag="mA")
            nc.tensor.matmul(
                M[:, 0:CH], lhsT=a_blk(t), rhs=D[:, 0:CH],
                start=True, stop=True,
            )
            prefetch[t] = M

        # na matmuls: PE runs them in its |a|-gated window before D1
        for t in range(NQT):
            nc.tensor.matmul(
                nbc[:, 16 + t : 17 + t], lhsT=absa_blk(t), rhs=ones8[:],
                start=True, stop=True,
            )
        with tc.tile_wait_until(0.0094):
            nc.vector.reciprocal(rna[:], nbc[:, 16:48])

        # early ACT reduce of tile-0 chunk 0 (c0 -> rs_a[0], c1 -> rs_d[0])
        nc.scalar.activation(
            trash_a[:, 0:CH], m0c0[:], _AF.Abs, bias=0.0,
            accum_out=rs_a[:, 0:1],
        )

        head_ctx.close()
        ring_b = ctx.enter_context(tc.tile_pool(name="m_psB", bufs=2, space="PSUM"))

        def emit_reduce(eng, m_ap, rs_tile, t):
            w = m_ap.shape[-1]
            if eng == "D":
                nc.vector.tensor_reduce(
                    out=rs_tile[:, t : t + 1], in_=m_ap, axis=_AX.X,
                    op=_OP.add, apply_absolute_value=True,
                )
            elif eng == "A":
                nc.scalar.activation(
                    trash_a[:, 0:w], m_ap, _AF.Abs, bias=0.0,
                    accum_out=rs_tile[:, t : t + 1],
                )

        # --- main loop ----------------------------------------------------
        for t in range(NQT):
            if t == 0:
                M = ring_a.tile([C, PSL], _F32, tag="mA")
                nc.tensor.matmul(
                    M[:, CH:PSL], lhsT=a_blk(0), rhs=D[:, CH:PSL],
                    start=True, stop=True,
                )
                nc.scalar.activation(
                    trash_a[:, 0:CH], M[:, CH:PSL], _AF.Abs, bias=0.0,
                    accum_out=rs_d[:, 0:1],
                )
                continue
            if t in prefetch:
                M = prefetch[t]
            else:
                pool, tag = (ring_a, "mA") if t % 4 in (0, 1) else (ring_b, "mB")
                M = pool.tile([C, PSL], _F32, tag=tag)
                nc.tensor.matmul(
                    M[:, 0:CH], lhsT=a_blk(t), rhs=D[:, 0:CH],
                    start=True, stop=True,
                )
            nc.tensor.matmul(
                M[:, CH:PSL], lhsT=a_blk(t), rhs=D[:, CH:PSL],
                start=True, stop=True,
            )
            e = lanes[t]
            emit_reduce(e, M[:], rs_d if e == "D" else rs_a, t)

            if t == 15:
                # first-half result: combine + scale + store while the
                # second half is still reducing
                nc.vector.tensor_tensor(out=sum1[:, 0:16], in0=rs_d[:, 0:16], in1=rs_a[:, 0:16], op=_OP.add)
                nc.vector.tensor_tensor(out=res[:, 0:16], in0=sum1[:, 0:16], in1=rna[:, 0:16], op=_OP.mult)
                nc.sync.dma_start(o_d[:, 0:16], res[:, 0:16])

        # --- tail ---------------------------------------------------------
        nc.vector.tensor_tensor(out=sum1[:, 16:32], in0=rs_d[:, 16:32], in1=rs_a[:, 16:32], op=_OP.add)
        nc.vector.tensor_tensor(out=res[:, 16:32], in0=sum1[:, 16:32], in1=rna[:, 16:32], op=_OP.mult)
        nc.sync.dma_start(o_d[:, 16:32], res[:, 16:32])


def _decls(nc):
    a_d = nc.dram_tensor("a_full", (C, HW), _FP8, kind="ExternalInput").ap()
    bc_d = nc.dram_tensor("bc", (C, 2 * PSL), _FP8, kind="ExternalInput").ap()
    oh_d = nc.dram_tensor("oh", (16, 16 * QT), _BF16, kind="ExternalInput").ap()
    id_d = nc.dram_tensor("ident", (C, C), _BF16, kind="ExternalInput").ap()
    o_d = nc.dram_tensor("out", (C, NQT), _F32, kind="ExternalOutput").ap()
    return a_d, bc_d, oh_d, id_d, o_d


def _build():
    nc = bacc.Bacc(
        "TRN2", target_bir_lowering=False, debug=False, num_devices=N_CORES
    )
    args = _decls(nc)
    with tile.TileContext(nc) as tc:
        _emit(tc, *args)
    nc.finalize()
    return nc


def build_single():
    """Single-core module for TimelineSim tracing."""
    nc = bacc.Bacc("TRN2", target_bir_lowering=False, debug=False)
    args = _decls(nc)
    with tile.TileContext(nc) as tc:
        _emit(tc, *args)
    nc.finalize()
    return nc


_NC_CACHE = {}


def _get_nc():
    if "nc" not in _NC_CACHE:
        _NC_CACHE["nc"] = _build()
    return _NC_CACHE["nc"]


def _fp8(x):
    import ml_dtypes

    return np.ascontiguousarray(x.astype(ml_dtypes.float8_e4m3))


def _bf16(x):
    import ml_dtypes

    return np.ascontiguousarray(x.astype(ml_dtypes.bfloat16))


def _in_maps(a, b, c):
    a = np.asarray(a, dtype=np.float32).reshape(B, C, HW)
    b = np.asarray(b, dtype=np.float32).reshape(B, C, HW)
    c = np.asarray(c, dtype=np.float32).reshape(B, C, HW)
    maps = []
    for core in range(N_CORES):
        bi, pi = divmod(core, 4)
        s0 = pi * PSL
        bc = np.concatenate(
            [
                b[bi, :, s0 : s0 + CH],
                c[bi, :, s0 : s0 + CH],
                b[bi, :, s0 + CH : s0 + PSL],
                c[bi, :, s0 + CH : s0 + PSL],
            ],
            axis=1,
        )
        maps.append(
            {
                "a_full": _fp8(a[bi]),
                "bc": _fp8(bc),
                "oh": _bf16(_onehot_const()),
                "ident": _bf16(np.eye(C, dtype=np.float32)),
            }
        )
    return maps


def _onehot_const():
    oh = np.zeros((16, 16 * QT), dtype=np.float32)
    for u in range(16):
        oh[u, u * QT : (u + 1) * QT] = 1.0
    return oh


def kernel(a, b, c):
    nc = _get_nc()
    res = bass_utils.run_bass_kernel_spmd(
        nc, _in_maps(a, b, c), core_ids=list(range(N_CORES))
    )
    total = np.float64(0.0)
    for core in range(N_CORES):
        total += np.sum(res.results[core]["out"], dtype=np.float64)
    return np.float32(total / (B * HW * HW))
